# revision 21
# baseline (speedup 1.0000x reference)
"""BondFastAttention Trainium2 kernel (self-contained).

Shapes (hardcoded from the problem spec):
  edge_attr [65536, 512] fp32, B=64 graphs x L=1024 bonds, HID=512, 8 heads x D=64.
  8 NeuronCores, data-parallel over graphs: G=8 graphs per core.

Device layout: feature-partition ("transposed") domain — features on partitions,
tokens on the free dim.  The four big GEMMs run in fp8e4m3 DoubleRow perf mode
with a hi/lo residual split (3 terms: hi*hi + lo*hi + hi*lo) so the quantization
error stays ~bf16-level.  Weights are pre-scaled by 32 on the host so both the
values and the residuals sit in fp8's normal range; every downstream consumer
folds the 1/32 back in (exp scale columns, stt scalar, and LayerNorm, which is
scale-invariant).  Stage C composes Mt = 32*(Wq + Wr diag(gk) Wv)^T on-device
per graph, so the V projection, Wr application, and +b_q additions all collapse
into one DoubleRow GEMM.  Softmax-over-D (partition dim) uses per-j-pair
selector matmuls for the segment sums, a DVE reciprocal, and a DMA
partition-broadcast for the per-token 1/s expansion; the softmax-weighted sums
are u = e*rbc on Pool followed by a fused scalar_tensor_tensor accumulate
against the live PSUM on DVE.
"""
import numpy as np

HID = 512
HEADS = 8
D = 64
B = 64
L = 1024
SCALE = D ** -0.5
EPS = 1e-5
NCORES = 8
G = B // NCORES          # graphs per core
NCH = HID // 128         # 4 feature chunks (2 heads each)
NT = L // 128            # 8 token chunks
WSC = 32.0               # host weight scale so fp8 hi/lo stay in normal range

# Accuracy/perf knobs (empirically validated; 3 = hi*hi + lo*hi + hi*lo)
QK_TERMS = 1
AP_TERMS = 3


def _build(apply_bo: bool, apply_affine: bool):
    import concourse.bass as bass
    from concourse import bacc
    import concourse.mybir as mybir
    from concourse.tile import TileContext

    F32 = mybir.dt.float32
    BF16 = mybir.dt.bfloat16
    FP8 = mybir.dt.float8e4
    AT = mybir.ActivationFunctionType
    OP = mybir.AluOpType
    DR = mybir.MatmulPerfMode.DoubleRow

    nc = bacc.Bacc()

    # One activation-table set (Exp/Ln/Copy/Identity/Relu) -> a single load.
    import concourse.bacc as _bacc_mod
    _orig_gat = _bacc_mod.get_activation_tables

    def _gat(arch):
        t = _orig_gat(arch)
        ours = {AT.Exp, AT.Ln, AT.Copy, AT.Relu, AT.Identity}
        out = {}
        for k, funcs in t.items():
            if k == "natural_log_exp_and_others":
                out[k] = funcs
            else:
                out[k] = {f for f in funcs if f not in ours}
        return out

    xthi = nc.dram_tensor("xthi", [HID, G * L], FP8, kind="ExternalInput")
    xtlo = nc.dram_tensor("xtlo", [HID, G * L], FP8, kind="ExternalInput")
    # DR-packed weights: per i-pair q a [128, 2*HID] tile, cols = (ktile, fout)
    wdr = {}
    for wn in ("q", "k"):
        for part in ("h", "l"):
            for q in range(2):
                name = f"w{wn}{part}{q}"
                wdr[name] = nc.dram_tensor(name, [128, 2 * HID], FP8,
                                           kind="ExternalInput")
    wot = nc.dram_tensor("wot", [HID, HID], BF16, kind="ExternalInput")
    wvn = nc.dram_tensor("wvn", [HID, HID], BF16, kind="ExternalInput")
    wqt32 = nc.dram_tensor("wqt32", [HID, HID], BF16, kind="ExternalInput")
    ident = nc.dram_tensor("ident", [128, 128], BF16, kind="ExternalInput")
    wrdup = nc.dram_tensor("wrdup", [128, D], BF16, kind="ExternalInput")
    segs4 = nc.dram_tensor("segs4", [128, 8 * NCH], BF16, kind="ExternalInput")
    wsa = nc.dram_tensor("wsa", [128, 1], F32, kind="ExternalInput")
    wbs32 = nc.dram_tensor("wbs32", [128, 1], F32, kind="ExternalInput")
    if apply_bo:
        bod = nc.dram_tensor("bo32", [1, HID], F32, kind="ExternalInput")
        onesd = nc.dram_tensor("ones1", [1, 128], F32, kind="ExternalInput")
    if apply_affine:
        lngd = nc.dram_tensor("ln_g", [128, HID], F32, kind="ExternalInput")
        lnbd = nc.dram_tensor("ln_b", [128, HID], F32, kind="ExternalInput")
    outd = nc.dram_tensor("out", [G * L, HID], F32, kind="ExternalOutput")

    with TileContext(nc) as tc:
        with tc.tile_pool(name="consts", bufs=1) as cp, \
             tc.tile_pool(name="big", bufs=1) as bp, \
             tc.tile_pool(name="small", bufs=2) as sp, \
             tc.tile_pool(name="qk", bufs=2, space="PSUM") as qkpool, \
             tc.tile_pool(name="pp", bufs=3, space="PSUM") as ppool, \
             tc.tile_pool(name="sps", bufs=1, space="PSUM") as spool:

            # ---- constants to SBUF ----
            w_sb = {}
            for name in wdr:
                w_sb[name] = cp.tile([128, 2 * HID], FP8, name=name, tag=name)
                nc.sync.dma_start(out=w_sb[name], in_=wdr[name].ap())
            wo_sb = [cp.tile([128, HID], BF16, name=f"wo{i}", tag=f"wo{i}")
                     for i in range(NCH)]
            wv_sb = [cp.tile([128, HID], BF16, name=f"wv{i}", tag=f"wv{i}")
                     for i in range(NCH)]
            wqt_sb = [cp.tile([128, HID], BF16, name=f"wqt{i}", tag=f"wqt{i}")
                      for i in range(NCH)]
            for i in range(NCH):
                nc.sync.dma_start(out=wo_sb[i], in_=wot.ap()[128 * i:128 * (i + 1), :])
                nc.sync.dma_start(out=wv_sb[i], in_=wvn.ap()[128 * i:128 * (i + 1), :])
                nc.sync.dma_start(out=wqt_sb[i], in_=wqt32.ap()[128 * i:128 * (i + 1), :])
            id_sb = cp.tile([128, 128], BF16)
            nc.sync.dma_start(out=id_sb, in_=ident.ap())
            wrdup_sb = cp.tile([128, D], BF16)
            nc.sync.dma_start(out=wrdup_sb, in_=wrdup.ap())
            segs_sb = cp.tile([128, 8 * NCH], BF16)
            nc.sync.dma_start(out=segs_sb, in_=segs4.ap())
            wsa_sb = cp.tile([128, 1], F32)
            nc.sync.dma_start(out=wsa_sb, in_=wsa.ap())
            wbs_sb = cp.tile([128, 1], F32)
            nc.sync.dma_start(out=wbs_sb, in_=wbs32.ap())
            eps_sb = cp.tile([128, 1], F32)
            nc.vector.memset(eps_sb, float(WSC * WSC * EPS))
            if apply_bo:
                ones1_sb = cp.tile([1, 128], F32)
                nc.sync.dma_start(out=ones1_sb, in_=onesd.ap())
                bo_sb = cp.tile([1, HID], F32)
                nc.sync.dma_start(out=bo_sb, in_=bod.ap())
            if apply_affine:
                lng_sb = cp.tile([128, HID], F32)
                nc.sync.dma_start(out=lng_sb, in_=lngd.ap())
                lnb_sb = cp.tile([128, HID], F32)
                nc.sync.dma_start(out=lnb_sb, in_=lnbd.ap())

            def dr_proj(pp, wn, xhi3, xlo3, j, nterms):
                """Accumulate nterms DR products into psum [128,1024] halves."""
                ops = [("h", xhi3), ("l", xhi3), ("h", xlo3)][:nterms]
                for n0 in (0, 512):
                    tot = 2 * len(ops)
                    k = 0
                    for part, xt3 in ops:
                        for q in range(2):
                            wt = w_sb[f"w{wn}{part}{q}"].rearrange(
                                "p (k f) -> p k f", k=2)
                            nc.tensor.matmul(
                                pp[:, n0:n0 + 512],
                                wt[:, :, 128 * j:128 * (j + 1)],
                                xt3[:, 2 * q:2 * q + 2, n0:n0 + 512],
                                start=(k == 0), stop=(k == tot - 1),
                                perf_mode=DR)
                            k += 1

            def softmax_chain(stage, e_all, rbc_all, qs_all, s_ps, accum_col, g):
                """recip -> bcast DMA -> Pool mul -> fused stt accumulate."""
                r_bf = sp.tile([8, L], BF16, name=f"rb{stage}{g}", tag="rbf",
                               bufs=3)
                for n0 in (0, 512):
                    rt = sp.tile([8, 512], F32, name=f"rt{stage}{g}{n0}",
                                 tag="rt")
                    nc.vector.reciprocal_approx_fast(out=rt, in_=s_ps[n0])
                    nc.vector.tensor_copy(out=r_bf[:, n0:n0 + 512], in_=rt)
                for j in range(NCH):
                    src = bass.AP(
                        tensor=r_bf.tensor,
                        offset=r_bf.offset + 2 * j * r_bf.ap[0][0],
                        ap=[[r_bf.ap[0][0], 2], [0, 64], [1, L]])
                    nc.sync.dma_start(
                        out=rbc_all[:, j * L:(j + 1) * L], in_=src)
                    uch = rbc_all[:, j * L:(j + 1) * L]
                    nc.gpsimd.tensor_tensor(
                        out=uch, in0=e_all[:, j * L:(j + 1) * L], in1=uch,
                        op=OP.mult)
                    nc.vector.scalar_tensor_tensor(
                        out=uch, in0=uch, scalar=1.0 / WSC,
                        in1=qs_all[:, j * L:(j + 1) * L],
                        op0=OP.mult, op1=OP.mult,
                        accum_out=accum_col[:, j:j + 1])

            st = {}

            def emit_A_gemms(g0):
                S = st[g0] = {}
                xh = bp.tile([128, NCH * L], FP8, name=f"xh{g0}", tag="xh",
                             bufs=3)
                xl = bp.tile([128, NCH * L], FP8, name=f"xl{g0}", tag="xl",
                             bufs=3)
                for t, dst in ((xthi, xh), (xtlo, xl)):
                    srcx = bass.AP(
                        tensor=t.ap().tensor, offset=g0 * L,
                        ap=[[G * L, 128], [128 * G * L, NCH], [1, L]])
                    nc.sync.dma_start(
                        out=dst.rearrange("p (i l) -> p i l", i=NCH), in_=srcx)
                S["xh3"] = xh.rearrange("p (i l) -> p i l", i=NCH)
                S["xl3"] = xl.rearrange("p (i l) -> p i l", i=NCH)
                S["e"] = bp.tile([128, NCH * L], BF16, name=f"ea{g0}",
                                 tag="e", bufs=3)
                S["rbc_a"] = bp.tile([128, NCH * L], BF16, name=f"ra{g0}",
                                     tag="rbc", bufs=3)
                S["qs"] = bp.tile([128, NCH * L], BF16, name=f"qs{g0}",
                                  tag="qs", bufs=2)
                S["kt"] = bp.tile([128, NCH * L], BF16, name=f"kt{g0}",
                                  tag="kt", bufs=2)
                S["gq"] = sp.tile([128, NCH], F32, name=f"gq{g0}", tag="gq")
                S["gqwb"] = sp.tile([128, NCH], F32, name=f"gqc{g0}", tag="gqc")
                S["s_a"] = {}
                for n0 in (0, 512):
                    S["s_a"][n0] = spool.tile(
                        [8, 512], F32, name=f"sa{g0}{n0}", tag="so",
                        padded_shape=[128, 512])
                for j in range(NCH):
                    pq = qkpool.tile([128, L], F32, name=f"pa{g0}{j}", tag="qk")
                    dr_proj(pq, "q", S["xh3"], S["xl3"], j, QK_TERMS)
                    qch = S["qs"][:, j * L:(j + 1) * L]
                    if j < 2:
                        nc.scalar.copy(out=qch, in_=pq)
                    else:
                        nc.vector.tensor_copy(out=qch, in_=pq)
                    ech = S["e"][:, j * L:(j + 1) * L]
                    nc.scalar.activation(out=ech, in_=qch, func=AT.Exp,
                                         scale=wsa_sb)
                    for n0 in (0, 512):
                        nc.tensor.matmul(
                            S["s_a"][n0], segs_sb[:, 8 * j:8 * j + 8],
                            ech[:, n0:n0 + 512],
                            start=(j == 0), stop=(j == NCH - 1))

            def emit_K_gemms(g0):
                S = st[g0]
                for j in range(NCH):
                    pk = qkpool.tile([128, L], F32, name=f"pk{g0}{j}", tag="qk")
                    dr_proj(pk, "k", S["xh3"], S["xl3"], j, QK_TERMS)
                    kch = S["kt"][:, j * L:(j + 1) * L]
                    if j < 2:
                        nc.scalar.copy(out=kch, in_=pk)
                    else:
                        nc.vector.tensor_copy(out=kch, in_=pk)

            def emit_recips(stage, s_ps, g):
                r_bf = sp.tile([8, L], BF16, name=f"rb{stage}{g}", tag="rbf",
                               bufs=3)
                for n0 in (0, 512):
                    rt = sp.tile([8, 512], F32, name=f"rt{stage}{g}{n0}",
                                 tag="rt")
                    nc.vector.reciprocal_approx_fast(out=rt, in_=s_ps[n0])
                    nc.vector.tensor_copy(out=r_bf[:, n0:n0 + 512], in_=rt)
                return r_bf

            def emit_chain(r_bf, e_all, rbc_all, qs_all, accum_col):
                for j in range(NCH):
                    src = bass.AP(
                        tensor=r_bf.tensor,
                        offset=r_bf.offset + 2 * j * r_bf.ap[0][0],
                        ap=[[r_bf.ap[0][0], 2], [0, 64], [1, L]])
                    nc.sync.dma_start(
                        out=rbc_all[:, j * L:(j + 1) * L], in_=src)
                    uch = rbc_all[:, j * L:(j + 1) * L]
                    nc.gpsimd.tensor_tensor(
                        out=uch, in0=e_all[:, j * L:(j + 1) * L], in1=uch,
                        op=OP.mult)
                    nc.vector.scalar_tensor_tensor(
                        out=uch, in0=uch, scalar=1.0 / WSC,
                        in1=qs_all[:, j * L:(j + 1) * L],
                        op0=OP.mult, op1=OP.mult,
                        accum_out=accum_col[:, j:j + 1])

            for i in range(G + 2):
                g0, g1, g2 = i, i - 1, i - 2

                # ---- step 1: stage B of g1 (inputs one iteration old) ----
                if 0 <= g1 < G:
                    S = st[g1]
                    S["eb"] = bp.tile([128, NCH * L], BF16, name=f"eb{g1}",
                                      tag="e", bufs=3)
                    S["rbc_b"] = bp.tile([128, NCH * L], BF16, name=f"rb{g1}",
                                         tag="rbc", bufs=3)
                    S["acc"] = sp.tile([128, NCH], F32, name=f"acc{g1}",
                                       tag="acc")
                    s_b = {}
                    for n0 in (0, 512):
                        s_b[n0] = spool.tile(
                            [8, 512], F32, name=f"sb{g1}{n0}", tag="so",
                            padded_shape=[128, 512])
                    for j in range(NCH):
                        ech = S["eb"][:, j * L:(j + 1) * L]
                        nc.scalar.activation(
                            out=ech, in_=S["kt"][:, j * L:(j + 1) * L],
                            func=AT.Exp, scale=S["gqwb"][:, j:j + 1])
                        for n0 in (0, 512):
                            nc.tensor.matmul(
                                s_b[n0], segs_sb[:, 8 * j:8 * j + 8],
                                ech[:, n0:n0 + 512],
                                start=(j == 0), stop=(j == NCH - 1))
                    S["rbf_b"] = emit_recips("b", s_b, g1)

                # ---- step 2: compose Mt of g2 (gk ready since last iter) ----
                if 0 <= g2 < G:
                    S = st[g2]
                    swrT = sp.tile([128, NCH * D], BF16, name=f"sw{g2}",
                                   tag="swr")
                    for j in range(NCH):
                        nc.vector.tensor_scalar_mul(
                            out=swrT[:, D * j:D * (j + 1)], in0=wrdup_sb,
                            scalar1=S["gk"][:, j:j + 1])
                    mthi = bp.tile([128, NCH * HID], FP8, name=f"mh{g2}",
                                   tag="mth", bufs=2)
                    mtlo = bp.tile([128, NCH * HID], FP8, name=f"ml{g2}",
                                   tag="mtl", bufs=2)
                    for ic in range(NCH):
                        mt_ps = ppool.tile([128, HID], F32, name=f"mt{g2}{ic}",
                                           tag="pp")
                        for h in range(HEADS):
                            reg = mt_ps[:, D * h:D * (h + 1)]
                            nc.tensor.matmul(
                                reg, id_sb, wqt_sb[ic][:, D * h:D * (h + 1)],
                                start=True, stop=False)
                            p0 = D * (h % 2)
                            nc.tensor.matmul(
                                reg,
                                wv_sb[h // 2][p0:p0 + D, 128 * ic:128 * (ic + 1)],
                                swrT[p0:p0 + D, D * (h // 2):D * (h // 2) + D],
                                start=False, stop=True)
                        hslc = mthi[:, HID * ic:HID * (ic + 1)]
                        nc.scalar.copy(out=hslc, in_=mt_ps)
                        nc.vector.scalar_tensor_tensor(
                            out=mtlo[:, HID * ic:HID * (ic + 1)], in0=mt_ps,
                            scalar=1.0, in1=hslc, op0=OP.mult, op1=OP.subtract)
                    S["mh3"] = mthi.rearrange("p (c f) -> p c f", c=NCH)
                    S["ml3"] = mtlo.rearrange("p (c f) -> p c f", c=NCH)

                # ---- step 3: Q GEMMs + exp + segs of g0 ----
                if g0 < G:
                    emit_A_gemms(g0)

                # ---- step 4: apply + relu of g2 ----
                if 0 <= g2 < G:
                    S = st[g2]
                    att_all = bp.tile([128, NCH * L], BF16, name=f"at{g2}",
                                      tag="att", bufs=2)
                    S["att"] = att_all
                    for j in range(NCH):
                        pc = qkpool.tile([128, L], F32, name=f"pc{g2}{j}",
                                         tag="qk")
                        ops = [(S["mh3"], S["xh3"]), (S["ml3"], S["xh3"]),
                               (S["mh3"], S["xl3"])][:AP_TERMS]
                        for n0 in (0, 512):
                            tot = 2 * len(ops)
                            k = 0
                            for mt3, xt3 in ops:
                                for q in range(2):
                                    nc.tensor.matmul(
                                        pc[:, n0:n0 + 512],
                                        mt3[:, 2 * q:2 * q + 2,
                                            128 * j:128 * (j + 1)],
                                        xt3[:, 2 * q:2 * q + 2, n0:n0 + 512],
                                        start=(k == 0), stop=(k == tot - 1),
                                        perf_mode=DR)
                                    k += 1
                        nc.scalar.activation(
                            out=att_all[:, j * L:(j + 1) * L], in_=pc,
                            func=AT.Relu)

                # ---- step 5: K GEMMs of g0 ----
                if g0 < G:
                    emit_K_gemms(g0)

                # ---- step 6: B chain of g1 ----
                if 0 <= g1 < G:
                    S = st[g1]
                    emit_chain(S["rbf_b"], S["eb"], S["rbc_b"], S["kt"],
                               S["acc"])
                    S["gk"] = sp.tile([128, NCH], F32, name=f"gk{g1}", tag="gk")
                    nc.vector.tensor_mul(out=S["gk"], in0=S["acc"],
                                         in1=S["gq"])

                # ---- step 7: Wo + LayerNorm of g2 ----
                if 0 <= g2 < G:
                    S = st[g2]
                    att_all = S["att"]
                    for t in range(NT):
                        o_ps = ppool.tile([128, HID], F32, name=f"o{g2}{t}",
                                          tag="pp")
                        for j in range(NCH):
                            nc.tensor.matmul(
                                o_ps,
                                att_all[:, j * L + 128 * t:j * L + 128 * (t + 1)],
                                wo_sb[j], start=(j == 0),
                                stop=(j == NCH - 1 and not apply_bo))
                        if apply_bo:
                            nc.tensor.matmul(o_ps, ones1_sb, bo_sb,
                                             start=False, stop=True)
                        stats = sp.tile([128, 6], F32, name=f"st{g2}{t}",
                                        tag="st")
                        nc.vector.bn_stats(out=stats, in_=o_ps)
                        mv = sp.tile([128, 2], F32, name=f"mv{g2}{t}", tag="mv")
                        nc.vector.bn_aggr(out=mv, in_=stats)
                        vf = sp.tile([128, 3], F32, name=f"vf{g2}{t}", tag="vf")
                        nc.vector.tensor_scalar_add(
                            out=vf[:, 0:1], in0=mv[:, 1:2],
                            scalar1=float(WSC * WSC * EPS))
                        nc.scalar.activation(out=vf[:, 1:2], in_=vf[:, 0:1],
                                             func=AT.Ln)
                        nc.scalar.activation(out=vf[:, 2:3], in_=vf[:, 1:2],
                                             func=AT.Exp, scale=-0.5)
                        nmr = sp.tile([128, 1], F32, name=f"nm{g2}{t}",
                                      tag="nmr")
                        nc.vector.tensor_scalar(
                            out=nmr, in0=mv[:, 0:1], scalar1=vf[:, 2:3],
                            scalar2=-1.0, op0=OP.mult, op1=OP.mult)
                        osb = sp.tile([128, HID], F32, name=f"ob{g2}{t}",
                                      tag="osb", bufs=3)
                        nc.scalar.activation(out=osb, in_=o_ps,
                                             func=AT.Identity,
                                             scale=vf[:, 2:3], bias=nmr)
                        if apply_affine:
                            nc.vector.tensor_mul(out=osb, in0=osb, in1=lng_sb)
                            nc.vector.tensor_add(out=osb, in0=osb, in1=lnb_sb)
                        nc.sync.dma_start(
                            out=outd.ap()[g2 * L + 128 * t:
                                          g2 * L + 128 * (t + 1), :],
                            in_=osb)

                # ---- step 8: A chain of g0 ----
                if g0 < G:
                    S = st[g0]
                    r_bf = emit_recips("a", S["s_a"], g0)
                    emit_chain(r_bf, S["e"], S["rbc_a"], S["qs"], S["gq"])
                    nc.vector.tensor_scalar_mul(out=S["gqwb"], in0=S["gq"],
                                                scalar1=wbs_sb)

    _bacc_mod.get_activation_tables = _gat
    try:
        nc.compile()
    finally:
        _bacc_mod.get_activation_tables = _orig_gat
    return nc


_NC_CACHE = {}


def _get_nc(apply_bo, apply_affine):
    key = (apply_bo, apply_affine)
    if key not in _NC_CACHE:
        _NC_CACHE[key] = _build(apply_bo, apply_affine)
    return _NC_CACHE[key]


def _fp8_split(a):
    import ml_dtypes
    f8 = ml_dtypes.float8_e4m3
    hi = a.astype(f8)
    lo = (a - hi.astype(np.float32)).astype(f8)
    return hi, lo


def _pack_dr(w32t):
    """[512,512] (in,out) -> per i-pair [128, 2*512] fp8 hi/lo DR tiles."""
    hi, lo = _fp8_split(w32t)
    out = {}
    for part, arr in (("h", hi), ("l", lo)):
        r = arr.reshape(4, 128, HID)
        for q in range(2):
            t = np.ascontiguousarray(
                r[2 * q:2 * q + 2].transpose(1, 0, 2).reshape(128, 2 * HID))
            out[f"{part}{q}"] = t
    return out


def _host_consts(Wq, Wk, Wv, Wr, w_alpha, w_beta, Wo, bo, ln_g, ln_b):
    import ml_dtypes
    bf = ml_dtypes.bfloat16

    wq_dr = _pack_dr(np.ascontiguousarray(WSC * Wq.T))
    wk_dr = _pack_dr(np.ascontiguousarray(WSC * Wk.T))
    common = {}
    for wn, drmap in (("q", wq_dr), ("k", wk_dr)):
        for k, v in drmap.items():
            common[f"w{wn}{k}"] = v

    common["wot"] = np.ascontiguousarray(Wo.T).astype(bf)
    common["wvn"] = np.ascontiguousarray(Wv).astype(bf)
    common["wqt32"] = np.ascontiguousarray(WSC * Wq.T).astype(bf)
    common["ident"] = np.eye(128, dtype=np.float32).astype(bf)
    wrdup = np.tile(WSC * Wr.T, (2, 1)).astype(np.float32)   # [128, 64]
    common["wrdup"] = wrdup.astype(bf)
    segs = np.zeros((128, 8 * NCH), np.float32)
    for j in range(NCH):
        for p in range(128):
            segs[p, 8 * j + 2 * j + p // 64] = 1.0
    common["segs4"] = segs.astype(bf)
    wa_col = (np.tile(w_alpha, 2) * SCALE / WSC).reshape(128, 1)
    common["wsa"] = wa_col.astype(np.float32)
    wb_col = (np.tile(w_beta, 2) * SCALE / WSC).reshape(128, 1)
    common["wbs32"] = wb_col.astype(np.float32)

    apply_bo = not np.allclose(bo, 0.0)
    apply_affine = not (np.allclose(ln_g, 1.0) and np.allclose(ln_b, 0.0))
    if apply_bo:
        common["bo32"] = (WSC * bo).reshape(1, HID).astype(np.float32)
        common["ones1"] = np.ones((1, 128), np.float32)
    if apply_affine:
        common["ln_g"] = np.tile(ln_g, (128, 1)).astype(np.float32)
        common["ln_b"] = np.tile(ln_b, (128, 1)).astype(np.float32)
    return common, apply_bo, apply_affine


def kernel(edge_attr, batch_scopes, Wq, Wk, Wv, Wr, w_alpha, w_beta, Wo, bo,
           ln_g, ln_b):
    from concourse import bass_utils

    edge_attr = np.asarray(edge_attr, dtype=np.float32)
    scopes = np.asarray(batch_scopes)
    Wq = np.asarray(Wq, np.float32); Wk = np.asarray(Wk, np.float32)
    Wv = np.asarray(Wv, np.float32); Wr = np.asarray(Wr, np.float32)
    Wo = np.asarray(Wo, np.float32)
    w_alpha = np.asarray(w_alpha, np.float32); w_beta = np.asarray(w_beta, np.float32)
    bo = np.asarray(bo, np.float32)
    ln_g = np.asarray(ln_g, np.float32); ln_b = np.asarray(ln_b, np.float32)

    assert np.all(scopes[:, 1] == L), "equal-length contiguous scopes expected"
    starts = scopes[:, 0].astype(np.int64)

    common, apply_bo, apply_affine = _host_consts(
        Wq, Wk, Wv, Wr, w_alpha, w_beta, Wo, bo, ln_g, ln_b)
    nc = _get_nc(apply_bo, apply_affine)

    in_maps = []
    for c in range(NCORES):
        rows = np.concatenate([
            np.arange(starts[c * G + g], starts[c * G + g] + L)
            for g in range(G)])
        xslab = np.ascontiguousarray(edge_attr[rows].T)   # [512, G*L]
        xhi, xlo = _fp8_split(xslab)
        in_maps.append({"xthi": xhi, "xtlo": xlo, **common})

    res = bass_utils.run_bass_kernel_spmd(nc, in_maps, core_ids=list(range(NCORES)))
    out = np.concatenate([r["out"] for r in res.results], axis=0)
    return out.astype(np.float32)


# revision 37
# speedup vs baseline: 1.1020x; 1.1020x over previous
"""BondFastAttention Trainium2 kernel (self-contained).

Shapes (hardcoded from the problem spec):
  edge_attr [65536, 512] fp32, B=64 graphs x L=1024 bonds, HID=512, 8 heads x D=64.
  8 NeuronCores, data-parallel over graphs: G=8 graphs per core.

Device layout: feature-partition ("transposed") domain — features on partitions,
tokens on the free dim.  The four big GEMMs run in fp8e4m3 DoubleRow perf mode
with a hi/lo residual split (3 terms: hi*hi + lo*hi + hi*lo) so the quantization
error stays ~bf16-level.  Weights are pre-scaled by 32 on the host so both the
values and the residuals sit in fp8's normal range; every downstream consumer
folds the 1/32 back in (exp scale columns, stt scalar, and LayerNorm, which is
scale-invariant).  Stage C composes Mt = 32*(Wq + Wr diag(gk) Wv)^T on-device
per graph, so the V projection, Wr application, and +b_q additions all collapse
into one DoubleRow GEMM.  Softmax-over-D (partition dim) uses per-j-pair
selector matmuls for the segment sums, a DVE reciprocal, and a DMA
partition-broadcast for the per-token 1/s expansion; the softmax-weighted sums
are u = e*rbc on Pool followed by a fused scalar_tensor_tensor accumulate
against the live PSUM on DVE.
"""
import numpy as np

HID = 512
HEADS = 8
D = 64
B = 64
L = 1024
SCALE = D ** -0.5
EPS = 1e-5
NCORES = 8
G = B // NCORES          # graphs per core
NCH = HID // 128         # 4 feature chunks (2 heads each)
NT = L // 128            # 8 token chunks
WSC = 32.0               # host weight scale so fp8 hi/lo stay in normal range

# Accuracy/perf knobs (empirically validated; 3 = hi*hi + lo*hi + hi*lo)
QK_TERMS = 2
AP_TERMS = 3


def _build(apply_bo: bool, apply_affine: bool):
    import concourse.bass as bass
    from concourse import bacc
    import concourse.mybir as mybir
    from concourse.tile import TileContext

    F32 = mybir.dt.float32
    BF16 = mybir.dt.bfloat16
    FP8 = mybir.dt.float8e4
    AT = mybir.ActivationFunctionType
    OP = mybir.AluOpType
    DR = mybir.MatmulPerfMode.DoubleRow

    nc = bacc.Bacc()

    # One activation-table set (Exp/Ln/Copy/Identity/Relu) -> a single load.
    import concourse.bacc as _bacc_mod
    _orig_gat = _bacc_mod.get_activation_tables

    def _gat(arch):
        t = _orig_gat(arch)
        ours = {AT.Exp, AT.Ln, AT.Copy, AT.Relu, AT.Identity}
        out = {}
        for k, funcs in t.items():
            if k == "natural_log_exp_and_others":
                out[k] = funcs
            else:
                out[k] = {f for f in funcs if f not in ours}
        return out

    xthi = nc.dram_tensor("xthi", [HID, G * L], FP8, kind="ExternalInput")
    xtlo = nc.dram_tensor("xtlo", [HID, G * L], FP8, kind="ExternalInput")
    # DR-packed weights: per i-pair q a [128, 2*HID] tile, cols = (ktile, fout)
    wdr = {}
    for wn in ("q", "k"):
        for part in ("h", "l"):
            for q in range(2):
                name = f"w{wn}{part}{q}"
                wdr[name] = nc.dram_tensor(name, [128, 2 * HID], FP8,
                                           kind="ExternalInput")
    wot = nc.dram_tensor("wot", [HID, HID], BF16, kind="ExternalInput")
    wvn = nc.dram_tensor("wvn", [HID, HID], BF16, kind="ExternalInput")
    wqt32 = nc.dram_tensor("wqt32", [HID, HID], BF16, kind="ExternalInput")
    ident = nc.dram_tensor("ident", [128, 128], BF16, kind="ExternalInput")
    wrdup = nc.dram_tensor("wrdup", [128, D], BF16, kind="ExternalInput")
    segs4 = nc.dram_tensor("segs4", [128, 8 * NCH], BF16, kind="ExternalInput")
    wsa = nc.dram_tensor("wsa", [128, 1], F32, kind="ExternalInput")
    wbs32 = nc.dram_tensor("wbs32", [128, 1], F32, kind="ExternalInput")
    if apply_bo:
        bod = nc.dram_tensor("bo32", [1, HID], F32, kind="ExternalInput")
        onesd = nc.dram_tensor("ones1", [1, 128], F32, kind="ExternalInput")
    if apply_affine:
        lngd = nc.dram_tensor("ln_g", [128, HID], F32, kind="ExternalInput")
        lnbd = nc.dram_tensor("ln_b", [128, HID], F32, kind="ExternalInput")
    outd = nc.dram_tensor("out", [G * L, HID], F32, kind="ExternalOutput")

    with TileContext(nc) as tc:
        with tc.tile_pool(name="consts", bufs=1) as cp, \
             tc.tile_pool(name="big", bufs=1) as bp, \
             tc.tile_pool(name="small", bufs=2) as sp, \
             tc.tile_pool(name="qk", bufs=2, space="PSUM") as qkpool, \
             tc.tile_pool(name="pp", bufs=4, space="PSUM") as ppool, \
             tc.tile_pool(name="sps", bufs=1, space="PSUM") as spool:

            # ---- constants to SBUF ----
            w_sb = {}
            for name in wdr:
                w_sb[name] = cp.tile([128, 2 * HID], FP8, name=name, tag=name)
                nc.sync.dma_start(out=w_sb[name], in_=wdr[name].ap())
            wo_sb = [cp.tile([128, HID], BF16, name=f"wo{i}", tag=f"wo{i}")
                     for i in range(NCH)]
            wv_sb = [cp.tile([128, HID], BF16, name=f"wv{i}", tag=f"wv{i}")
                     for i in range(NCH)]
            wqt_sb = [cp.tile([128, HID], BF16, name=f"wqt{i}", tag=f"wqt{i}")
                      for i in range(NCH)]
            for i in range(NCH):
                nc.sync.dma_start(out=wo_sb[i], in_=wot.ap()[128 * i:128 * (i + 1), :])
                nc.sync.dma_start(out=wv_sb[i], in_=wvn.ap()[128 * i:128 * (i + 1), :])
                nc.sync.dma_start(out=wqt_sb[i], in_=wqt32.ap()[128 * i:128 * (i + 1), :])
            id_sb = cp.tile([128, 128], BF16)
            nc.sync.dma_start(out=id_sb, in_=ident.ap())
            wrdup_sb = cp.tile([128, D], BF16)
            nc.sync.dma_start(out=wrdup_sb, in_=wrdup.ap())
            segs_sb = cp.tile([128, 8 * NCH], BF16)
            nc.sync.dma_start(out=segs_sb, in_=segs4.ap())
            wsa_sb = cp.tile([128, 1], F32)
            nc.sync.dma_start(out=wsa_sb, in_=wsa.ap())
            wbs_sb = cp.tile([128, 1], F32)
            nc.sync.dma_start(out=wbs_sb, in_=wbs32.ap())
            eps_sb = cp.tile([128, 1], F32)
            nc.vector.memset(eps_sb, float(WSC * WSC * EPS))
            if apply_bo:
                ones1_sb = cp.tile([1, 128], F32)
                nc.sync.dma_start(out=ones1_sb, in_=onesd.ap())
                bo_sb = cp.tile([1, HID], F32)
                nc.sync.dma_start(out=bo_sb, in_=bod.ap())
            if apply_affine:
                lng_sb = cp.tile([128, HID], F32)
                nc.sync.dma_start(out=lng_sb, in_=lngd.ap())
                lnb_sb = cp.tile([128, HID], F32)
                nc.sync.dma_start(out=lnb_sb, in_=lnbd.ap())

            def dr_proj(pp, wn, xhi3, xlo3, j, nterms):
                """Accumulate nterms DR products into psum [128,1024] halves."""
                ops = [("h", xhi3), ("l", xhi3), ("h", xlo3)][:nterms]
                for n0 in (0, 512):
                    tot = 2 * len(ops)
                    k = 0
                    for part, xt3 in ops:
                        for q in range(2):
                            wt = w_sb[f"w{wn}{part}{q}"].rearrange(
                                "p (k f) -> p k f", k=2)
                            nc.tensor.matmul(
                                pp[:, n0:n0 + 512],
                                wt[:, :, 128 * j:128 * (j + 1)],
                                xt3[:, 2 * q:2 * q + 2, n0:n0 + 512],
                                start=(k == 0), stop=(k == tot - 1),
                                perf_mode=DR)
                            k += 1

            def softmax_chain(stage, e_all, rbc_all, qs_all, s_ps, accum_col, g):
                """recip -> bcast DMA -> Pool mul -> fused stt accumulate."""
                r_bf = sp.tile([8, L], BF16, name=f"rb{stage}{g}", tag="rbf",
                               bufs=3)
                for n0 in (0, 512):
                    rt = sp.tile([8, 512], F32, name=f"rt{stage}{g}{n0}",
                                 tag="rt")
                    nc.vector.reciprocal_approx_fast(out=rt, in_=s_ps[n0])
                    nc.gpsimd.tensor_copy(out=r_bf[:, n0:n0 + 512], in_=rt)
                for j in range(NCH):
                    src = bass.AP(
                        tensor=r_bf.tensor,
                        offset=r_bf.offset + 2 * j * r_bf.ap[0][0],
                        ap=[[r_bf.ap[0][0], 2], [0, 64], [1, L]])
                    nc.sync.dma_start(
                        out=rbc_all[:, j * L:(j + 1) * L], in_=src)
                    uch = rbc_all[:, j * L:(j + 1) * L]
                    nc.gpsimd.tensor_tensor(
                        out=uch, in0=e_all[:, j * L:(j + 1) * L], in1=uch,
                        op=OP.mult)
                    nc.vector.scalar_tensor_tensor(
                        out=uch, in0=uch, scalar=1.0 / WSC,
                        in1=qs_all[:, j * L:(j + 1) * L],
                        op0=OP.mult, op1=OP.mult,
                        accum_out=accum_col[:, j:j + 1])

            def _make_emit_qj(g0):
                def emit_qj(j):
                    S = st[g0]
                    pq = qkpool.tile([128, L], F32, name=f"pa{g0}{j}",
                                     tag="qk")
                    dr_proj(pq, "q", S["xh3"], S["xl3"], j, QK_TERMS)
                    qch = S["qs"][:, j * L:(j + 1) * L]
                    if j < 2:
                        nc.scalar.copy(out=qch, in_=pq)
                    else:
                        nc.vector.tensor_copy(out=qch, in_=pq)
                    ech = S["e"][:, j * L:(j + 1) * L]
                    nc.scalar.activation(out=ech, in_=qch, func=AT.Exp,
                                         scale=wsa_sb)
                    for n0 in (0, 512):
                        nc.tensor.matmul(
                            S["s_a"][n0], segs_sb[:, 8 * j:8 * j + 8],
                            ech[:, n0:n0 + 512],
                            start=(j == 0), stop=(j == NCH - 1))
                return emit_qj

            def _make_emit_ot(g2):
                ctx = {}
                def emit_ot(t):
                    S = st[g2]
                    att_all = S["att"]
                return emit_ot

            st = {}

            def emit_A_gemms(g0):
                S = st[g0] = {}
                xh = bp.tile([128, NCH * L], FP8, name=f"xh{g0}", tag="xh",
                             bufs=3)
                xl = bp.tile([128, NCH * L], FP8, name=f"xl{g0}", tag="xl",
                             bufs=3)
                for t, dst in ((xthi, xh), (xtlo, xl)):
                    srcx = bass.AP(
                        tensor=t.ap().tensor, offset=g0 * L,
                        ap=[[G * L, 128], [128 * G * L, NCH], [1, L]])
                    nc.sync.dma_start(
                        out=dst.rearrange("p (i l) -> p i l", i=NCH), in_=srcx)
                S["xh3"] = xh.rearrange("p (i l) -> p i l", i=NCH)
                S["xl3"] = xl.rearrange("p (i l) -> p i l", i=NCH)
                S["e"] = bp.tile([128, NCH * L], BF16, name=f"ea{g0}",
                                 tag="e", bufs=3)
                S["rbc_a"] = bp.tile([128, NCH * L], BF16, name=f"ra{g0}",
                                     tag="rbc", bufs=3)
                S["qs"] = bp.tile([128, NCH * L], BF16, name=f"qs{g0}",
                                  tag="qs", bufs=2)
                S["kt"] = bp.tile([128, NCH * L], BF16, name=f"kt{g0}",
                                  tag="kt", bufs=2)
                S["gq"] = sp.tile([128, NCH], F32, name=f"gq{g0}", tag="gq")
                S["gqwb"] = sp.tile([128, NCH], F32, name=f"gqc{g0}", tag="gqc")
                S["s_a"] = {}
                for n0 in (0, 512):
                    S["s_a"][n0] = ppool.tile(
                        [8, 512], F32, name=f"sa{g0}{n0}", tag="pp",
                        padded_shape=[128, 512])
                S["emit_qj"] = _make_emit_qj(g0)

            def emit_K_gemms(g0):
                S = st[g0]
                for j in range(NCH):
                    pk = qkpool.tile([128, L], F32, name=f"pk{g0}{j}", tag="qk")
                    dr_proj(pk, "k", S["xh3"], S["xl3"], j, QK_TERMS)
                    kch = S["kt"][:, j * L:(j + 1) * L]
                    if j < 2:
                        nc.scalar.copy(out=kch, in_=pk)
                    else:
                        nc.vector.tensor_copy(out=kch, in_=pk)

            def emit_recips(stage, s_ps, g):
                r_bf = sp.tile([8, L], BF16, name=f"rb{stage}{g}", tag="rbf",
                               bufs=3)
                for n0 in (0, 512):
                    rt = sp.tile([8, 512], F32, name=f"rt{stage}{g}{n0}",
                                 tag="rt")
                    nc.vector.reciprocal_approx_fast(out=rt, in_=s_ps[n0])
                    nc.gpsimd.tensor_copy(out=r_bf[:, n0:n0 + 512], in_=rt)
                return r_bf

            def emit_chain(r_bf, e_all, rbc_all, qs_all, accum_col):
                for j in range(NCH):
                    for n0 in (0, 512):
                        src = bass.AP(
                            tensor=r_bf.tensor,
                            offset=r_bf.offset + 2 * j * r_bf.ap[0][0] + n0,
                            ap=[[r_bf.ap[0][0], 2], [0, 64], [1, 512]])
                        nc.sync.dma_start(
                            out=rbc_all[:, j * L + n0:j * L + n0 + 512],
                            in_=src)
                        uch = rbc_all[:, j * L + n0:j * L + n0 + 512]
                        nc.gpsimd.tensor_tensor(
                            out=uch,
                            in0=e_all[:, j * L + n0:j * L + n0 + 512],
                            in1=uch, op=OP.mult)
                    uj = rbc_all[:, j * L:(j + 1) * L]
                    nc.vector.scalar_tensor_tensor(
                        out=uj, in0=uj, scalar=1.0 / WSC,
                        in1=qs_all[:, j * L:(j + 1) * L],
                        op0=OP.mult, op1=OP.mult,
                        accum_out=accum_col[:, j:j + 1])

            for i in range(G + 4):
                g0, g1, g2, g3 = i, i - 1, i - 2, i - 3

                # ---- step 0: Wo + LayerNorm of g3 (runs first: inputs ready) ----
                if 0 <= g3 < G:
                    S = st[g3]
                    att_all = S["att"]
                    for t in range(NT):
                        o_ps = ppool.tile([128, HID], F32, name=f"o{g3}{t}",
                                          tag="pp")
                        for j in range(NCH):
                            nc.tensor.matmul(
                                o_ps,
                                att_all[:, j * L + 128 * t:j * L + 128 * (t + 1)],
                                wo_sb[j], start=(j == 0),
                                stop=(j == NCH - 1 and not apply_bo))
                        if apply_bo:
                            nc.tensor.matmul(o_ps, ones1_sb, bo_sb,
                                             start=False, stop=True)
                        stats = sp.tile([128, 6], F32, name=f"st{g3}{t}",
                                        tag="st")
                        nc.vector.bn_stats(out=stats, in_=o_ps)
                        mv = sp.tile([128, 2], F32, name=f"mv{g3}{t}", tag="mv")
                        nc.vector.bn_aggr(out=mv, in_=stats)
                        vf = sp.tile([128, 3], F32, name=f"vf{g3}{t}", tag="vf")
                        nc.vector.tensor_scalar_add(
                            out=vf[:, 0:1], in0=mv[:, 1:2],
                            scalar1=float(WSC * WSC * EPS))
                        nc.scalar.activation(out=vf[:, 1:2], in_=vf[:, 0:1],
                                             func=AT.Ln)
                        nc.scalar.activation(out=vf[:, 2:3], in_=vf[:, 1:2],
                                             func=AT.Exp, scale=-0.5)
                        nmr = sp.tile([128, 1], F32, name=f"nm{g3}{t}",
                                      tag="nmr")
                        nc.vector.tensor_scalar(
                            out=nmr, in0=mv[:, 0:1], scalar1=vf[:, 2:3],
                            scalar2=-1.0, op0=OP.mult, op1=OP.mult)
                        if t % 2 == 0:
                            osb_pair = sp.tile([128, 2 * HID], F32,
                                               name=f"ob{g3}{t}", tag="osb",
                                               bufs=3)
                        osb = osb_pair[:, HID * (t % 2):HID * (t % 2 + 1)]
                        nc.scalar.activation(out=osb, in_=o_ps,
                                             func=AT.Identity,
                                             scale=vf[:, 2:3], bias=nmr)
                        if apply_affine:
                            nc.vector.tensor_mul(out=osb, in0=osb, in1=lng_sb)
                            nc.vector.tensor_add(out=osb, in0=osb, in1=lnb_sb)
                        if t % 2 == 1:
                            dst = outd.ap()[g3 * L + 128 * (t - 1):
                                            g3 * L + 128 * (t + 1), :]
                            nc.sync.dma_start(
                                out=dst,
                                in_=osb_pair.rearrange(
                                    "p (b h) -> b p h", b=2))



                # ---- step 1: stage B of g1 (inputs one iteration old) ----
                if 0 <= g1 < G:
                    S = st[g1]
                    S["eb"] = bp.tile([128, NCH * L], BF16, name=f"eb{g1}",
                                      tag="e", bufs=3)
                    S["rbc_b"] = bp.tile([128, NCH * L], BF16, name=f"rb{g1}",
                                         tag="rbc", bufs=3)
                    S["acc"] = sp.tile([128, NCH], F32, name=f"acc{g1}",
                                       tag="acc")
                    s_b = {}
                    for n0 in (0, 512):
                        s_b[n0] = ppool.tile(
                            [8, 512], F32, name=f"sb{g1}{n0}", tag="pp",
                            padded_shape=[128, 512])
                    for j in range(NCH):
                        ech = S["eb"][:, j * L:(j + 1) * L]
                        nc.scalar.activation(
                            out=ech, in_=S["kt"][:, j * L:(j + 1) * L],
                            func=AT.Exp, scale=S["gqwb"][:, j:j + 1])
                        for n0 in (0, 512):
                            nc.tensor.matmul(
                                s_b[n0], segs_sb[:, 8 * j:8 * j + 8],
                                ech[:, n0:n0 + 512],
                                start=(j == 0), stop=(j == NCH - 1))
                    S["rbf_b"] = emit_recips("b", s_b, g1)

                # ---- step 2: compose Mt of g2 (gk ready since last iter) ----
                if 0 <= g2 < G:
                    S = st[g2]
                    swrT = sp.tile([128, NCH * D], BF16, name=f"sw{g2}",
                                   tag="swr")
                    for j in range(NCH):
                        nc.vector.tensor_scalar_mul(
                            out=swrT[:, D * j:D * (j + 1)], in0=wrdup_sb,
                            scalar1=S["gk"][:, j:j + 1])
                    mthi = bp.tile([128, NCH * HID], FP8, name=f"mh{g2}",
                                   tag="mth", bufs=2)
                    mtlo = bp.tile([128, NCH * HID], FP8, name=f"ml{g2}",
                                   tag="mtl", bufs=2)
                    for ic in range(NCH):
                        mt_ps = ppool.tile([128, HID], F32, name=f"mt{g2}{ic}",
                                           tag="pp")
                        for h in range(HEADS):
                            reg = mt_ps[:, D * h:D * (h + 1)]
                            nc.tensor.matmul(
                                reg, id_sb, wqt_sb[ic][:, D * h:D * (h + 1)],
                                start=True, stop=False)
                            p0 = D * (h % 2)
                            nc.tensor.matmul(
                                reg,
                                wv_sb[h // 2][p0:p0 + D, 128 * ic:128 * (ic + 1)],
                                swrT[p0:p0 + D, D * (h // 2):D * (h // 2) + D],
                                start=False, stop=True)
                        hslc = mthi[:, HID * ic:HID * (ic + 1)]
                        nc.scalar.copy(out=hslc, in_=mt_ps)
                        nc.vector.scalar_tensor_tensor(
                            out=mtlo[:, HID * ic:HID * (ic + 1)], in0=mt_ps,
                            scalar=1.0, in1=hslc, op0=OP.mult, op1=OP.subtract)
                    S["mh3"] = mthi.rearrange("p (c f) -> p c f", c=NCH)
                    S["ml3"] = mtlo.rearrange("p (c f) -> p c f", c=NCH)

                # ---- step 3: Q GEMMs + exp + segs of g0 ----
                if g0 < G:
                    emit_A_gemms(g0)
                    for j in range(NCH):
                        st[g0]["emit_qj"](j)

                # ---- step 4: apply + relu of g2 ----
                if 0 <= g2 < G:
                    S = st[g2]
                    att_all = bp.tile([128, NCH * L], BF16, name=f"at{g2}",
                                      tag="att", bufs=2)
                    S["att"] = att_all
                    for j in range(NCH):
                        pc = qkpool.tile([128, L], F32, name=f"pc{g2}{j}",
                                         tag="qk")
                        ops = [(S["mh3"], S["xh3"]), (S["ml3"], S["xh3"]),
                               (S["mh3"], S["xl3"])][:AP_TERMS]
                        for n0 in (0, 512):
                            tot = 2 * len(ops)
                            k = 0
                            for mt3, xt3 in ops:
                                for q in range(2):
                                    nc.tensor.matmul(
                                        pc[:, n0:n0 + 512],
                                        mt3[:, 2 * q:2 * q + 2,
                                            128 * j:128 * (j + 1)],
                                        xt3[:, 2 * q:2 * q + 2, n0:n0 + 512],
                                        start=(k == 0), stop=(k == tot - 1),
                                        perf_mode=DR)
                                    k += 1
                        nc.scalar.activation(
                            out=att_all[:, j * L:(j + 1) * L], in_=pc,
                            func=AT.Relu)

                # ---- step 5: K GEMMs of g0 ----
                if g0 < G:
                    emit_K_gemms(g0)

                # ---- step 6: B chain of g1 ----
                if 0 <= g1 < G:
                    S = st[g1]
                    emit_chain(S["rbf_b"], S["eb"], S["rbc_b"], S["kt"],
                               S["acc"])
                    S["gk"] = sp.tile([128, NCH], F32, name=f"gk{g1}", tag="gk")
                    nc.vector.tensor_mul(out=S["gk"], in0=S["acc"],
                                         in1=S["gq"])

                # ---- step 7: Wo + LayerNorm of g2 ----
                if 0 <= g2 < G:
                    emit_ot = _make_emit_ot(g2)
                    for t in range(NT):
                        emit_ot(t)

                # ---- step 8: A chain of g0 ----
                if g0 < G:
                    S = st[g0]
                    r_bf = emit_recips("a", S["s_a"], g0)
                    emit_chain(r_bf, S["e"], S["rbc_a"], S["qs"], S["gq"])
                    nc.vector.tensor_scalar_mul(out=S["gqwb"], in0=S["gq"],
                                                scalar1=wbs_sb)

    _bacc_mod.get_activation_tables = _gat
    try:
        nc.compile()
    finally:
        _bacc_mod.get_activation_tables = _orig_gat
    return nc


_NC_CACHE = {}


def _get_nc(apply_bo, apply_affine):
    key = (apply_bo, apply_affine)
    if key not in _NC_CACHE:
        _NC_CACHE[key] = _build(apply_bo, apply_affine)
    return _NC_CACHE[key]


def _fp8_split(a):
    import ml_dtypes
    f8 = ml_dtypes.float8_e4m3
    hi = a.astype(f8)
    lo = (a - hi.astype(np.float32)).astype(f8)
    return hi, lo


def _pack_dr(w32t):
    """[512,512] (in,out) -> per i-pair [128, 2*512] fp8 hi/lo DR tiles."""
    hi, lo = _fp8_split(w32t)
    out = {}
    for part, arr in (("h", hi), ("l", lo)):
        r = arr.reshape(4, 128, HID)
        for q in range(2):
            t = np.ascontiguousarray(
                r[2 * q:2 * q + 2].transpose(1, 0, 2).reshape(128, 2 * HID))
            out[f"{part}{q}"] = t
    return out


def _host_consts(Wq, Wk, Wv, Wr, w_alpha, w_beta, Wo, bo, ln_g, ln_b):
    import ml_dtypes
    bf = ml_dtypes.bfloat16

    wq_dr = _pack_dr(np.ascontiguousarray(WSC * Wq.T))
    wk_dr = _pack_dr(np.ascontiguousarray(WSC * Wk.T))
    common = {}
    for wn, drmap in (("q", wq_dr), ("k", wk_dr)):
        for k, v in drmap.items():
            common[f"w{wn}{k}"] = v

    common["wot"] = np.ascontiguousarray(Wo.T).astype(bf)
    common["wvn"] = np.ascontiguousarray(Wv).astype(bf)
    common["wqt32"] = np.ascontiguousarray(WSC * Wq.T).astype(bf)
    common["ident"] = np.eye(128, dtype=np.float32).astype(bf)
    wrdup = np.tile(WSC * Wr.T, (2, 1)).astype(np.float32)   # [128, 64]
    common["wrdup"] = wrdup.astype(bf)
    segs = np.zeros((128, 8 * NCH), np.float32)
    for j in range(NCH):
        for p in range(128):
            segs[p, 8 * j + 2 * j + p // 64] = 1.0
    common["segs4"] = segs.astype(bf)
    wa_col = (np.tile(w_alpha, 2) * SCALE / WSC).reshape(128, 1)
    common["wsa"] = wa_col.astype(np.float32)
    wb_col = (np.tile(w_beta, 2) * SCALE / WSC).reshape(128, 1)
    common["wbs32"] = wb_col.astype(np.float32)

    apply_bo = not np.allclose(bo, 0.0)
    apply_affine = not (np.allclose(ln_g, 1.0) and np.allclose(ln_b, 0.0))
    if apply_bo:
        common["bo32"] = (WSC * bo).reshape(1, HID).astype(np.float32)
        common["ones1"] = np.ones((1, 128), np.float32)
    if apply_affine:
        common["ln_g"] = np.tile(ln_g, (128, 1)).astype(np.float32)
        common["ln_b"] = np.tile(ln_b, (128, 1)).astype(np.float32)
    return common, apply_bo, apply_affine


def kernel(edge_attr, batch_scopes, Wq, Wk, Wv, Wr, w_alpha, w_beta, Wo, bo,
           ln_g, ln_b):
    from concourse import bass_utils

    edge_attr = np.asarray(edge_attr, dtype=np.float32)
    scopes = np.asarray(batch_scopes)
    Wq = np.asarray(Wq, np.float32); Wk = np.asarray(Wk, np.float32)
    Wv = np.asarray(Wv, np.float32); Wr = np.asarray(Wr, np.float32)
    Wo = np.asarray(Wo, np.float32)
    w_alpha = np.asarray(w_alpha, np.float32); w_beta = np.asarray(w_beta, np.float32)
    bo = np.asarray(bo, np.float32)
    ln_g = np.asarray(ln_g, np.float32); ln_b = np.asarray(ln_b, np.float32)

    assert np.all(scopes[:, 1] == L), "equal-length contiguous scopes expected"
    starts = scopes[:, 0].astype(np.int64)

    common, apply_bo, apply_affine = _host_consts(
        Wq, Wk, Wv, Wr, w_alpha, w_beta, Wo, bo, ln_g, ln_b)
    nc = _get_nc(apply_bo, apply_affine)

    in_maps = []
    for c in range(NCORES):
        rows = np.concatenate([
            np.arange(starts[c * G + g], starts[c * G + g] + L)
            for g in range(G)])
        xslab = np.ascontiguousarray(edge_attr[rows].T)   # [512, G*L]
        xhi, xlo = _fp8_split(xslab)
        in_maps.append({"xthi": xhi, "xtlo": xlo, **common})

    res = bass_utils.run_bass_kernel_spmd(nc, in_maps, core_ids=list(range(NCORES)))
    out = np.concatenate([r["out"] for r in res.results], axis=0)
    return out.astype(np.float32)                # ---- step 7: Wo + LayerNorm of g2 ----
                if 0 <= g2 < G:
                    emit_ot = _make_emit_ot(g2)
                    for t in range(NT):
                        emit_ot(t)

                # ---- step 8: A chain of g0 ----
                if g0 < G:
                    S = st[g0]
                    r_bf = emit_recips("a", S["s_a"], g0)
                    emit_chain(r_bf, S["e"], S["rbc_a"], S["qs"], S["gq"])
                    nc.vector.tensor_scalar_mul(out=S["gqwb"], in0=S["gq"],
                                                scalar1=wbs_sb)

    _bacc_mod.get_activation_tables = _gat
    try:
        nc.compile()
    finally:
        _bacc_mod.get_activation_tables = _orig_gat
    return nc


_NC_CACHE = {}


def _get_nc(apply_bo, apply_affine):
    key = (apply_bo, apply_affine)
    if key not in _NC_CACHE:
        _NC_CACHE[key] = _build(apply_bo, apply_affine)
    return _NC_CACHE[key]


def _fp8_split(a):
    import ml_dtypes
    f8 = ml_dtypes.float8_e4m3
    hi = a.astype(f8)
    lo = (a - hi.astype(np.float32)).astype(f8)
    return hi, lo


def _pack_dr(w32t):
    """[512,512] (in,out) -> per i-pair [128, 2*512] fp8 hi/lo DR tiles."""
    hi, lo = _fp8_split(w32t)
    out = {}
    for part, arr in (("h", hi), ("l", lo)):
        r = arr.reshape(4, 128, HID)
        for q in range(2):
            t = np.ascontiguousarray(
                r[2 * q:2 * q + 2].transpose(1, 0, 2).reshape(128, 2 * HID))
            out[f"{part}{q}"] = t
    return out


def _host_consts(Wq, Wk, Wv, Wr, w_alpha, w_beta, Wo, bo, ln_g, ln_b):
    import ml_dtypes
    bf = ml_dtypes.bfloat16

    wq_dr = _pack_dr(np.ascontiguousarray(WSC * Wq.T))
    wk_dr = _pack_dr(np.ascontiguousarray(WSC * Wk.T))
    common = {}
    for wn, drmap in (("q", wq_dr), ("k", wk_dr)):
        for k, v in drmap.items():
            common[f"w{wn}{k}"] = v

    common["wot"] = np.ascontiguousarray(Wo.T).astype(bf)
    common["wvn"] = np.ascontiguousarray(Wv).astype(bf)
    common["wqt32"] = np.ascontiguousarray(WSC * Wq.T).astype(bf)
    common["ident"] = np.eye(128, dtype=np.float32).astype(bf)
    wrdup = np.tile(WSC * Wr.T, (2, 1)).astype(np.float32)   # [128, 64]
    common["wrdup"] = wrdup.astype(bf)
    segs = np.zeros((128, 8 * NCH), np.float32)
    for j in range(NCH):
        for p in range(128):
            segs[p, 8 * j + 2 * j + p // 64] = 1.0
    common["segs4"] = segs.astype(bf)
    wa_col = (np.tile(w_alpha, 2) * SCALE / WSC).reshape(128, 1)
    common["wsa"] = wa_col.astype(np.float32)
    wb_col = (np.tile(w_beta, 2) * SCALE / WSC).reshape(128, 1)
    common["wbs32"] = wb_col.astype(np.float32)

    apply_bo = not np.allclose(bo, 0.0)
    apply_affine = not (np.allclose(ln_g, 1.0) and np.allclose(ln_b, 0.0))
    if apply_bo:
        common["bo32"] = (WSC * bo).reshape(1, HID).astype(np.float32)
        common["ones1"] = np.ones((1, 128), np.float32)
    if apply_affine:
        common["ln_g"] = np.tile(ln_g, (128, 1)).astype(np.float32)
        common["ln_b"] = np.tile(ln_b, (128, 1)).astype(np.float32)
    return common, apply_bo, apply_affine


def kernel(edge_attr, batch_scopes, Wq, Wk, Wv, Wr, w_alpha, w_beta, Wo, bo,
           ln_g, ln_b):
    from concourse import bass_utils

    edge_attr = np.asarray(edge_attr, dtype=np.float32)
    scopes = np.asarray(batch_scopes)
    Wq = np.asarray(Wq, np.float32); Wk = np.asarray(Wk, np.float32)
    Wv = np.asarray(Wv, np.float32); Wr = np.asarray(Wr, np.float32)
    Wo = np.asarray(Wo, np.float32)
    w_alpha = np.asarray(w_alpha, np.float32); w_beta = np.asarray(w_beta, np.float32)
    bo = np.asarray(bo, np.float32)
    ln_g = np.asarray(ln_g, np.float32); ln_b = np.asarray(ln_b, np.float32)

    assert np.all(scopes[:, 1] == L), "equal-length contiguous scopes expected"
    starts = scopes[:, 0].astype(np.int64)

    common, apply_bo, apply_affine = _host_consts(
        Wq, Wk, Wv, Wr, w_alpha, w_beta, Wo, bo, ln_g, ln_b)
    nc = _get_nc(apply_bo, apply_affine)

    in_maps = []
    for c in range(NCORES):
        rows = np.concatenate([
            np.arange(starts[c * G + g], starts[c * G + g] + L)
            for g in range(G)])
        xslab = np.ascontiguousarray(edge_attr[rows].T)   # [512, G*L]
        xhi, xlo = _fp8_split(xslab)
        in_maps.append({"xthi": xhi, "xtlo": xlo, **common})

    res = bass_utils.run_bass_kernel_spmd(nc, in_maps, core_ids=list(range(NCORES)))
    out = np.concatenate([r["out"] for r in res.results], axis=0)
    return out.astype(np.float32)
